# revision 19
# baseline (speedup 1.0000x reference)
"""Trainium2 Bass kernel for nn_ContinuousEmbedding (histogram binning + distance-
weighted embedding mix).

Math: for each scalar x[b,f], the reference computes bucket index
idx = #{j in 1..63 : x > low[j]} and returns
    out[b,f,:] = sum_k weight[k,:] / (|idx-k|+1)  =  T[idx,:]
where T = S @ weight, S[i,k] = 1/(|i-k|+1) is a fixed 64x64 matrix.

T[idx] telescopes over compare signs s_j = sign(x - low[j]) (s_0 = +1 since
low[0] = -inf):
    T[idx] = sum_j s_j * V2[j],  V2[0] = (T[0]+T[63])/2, V2[j] = (T[j]-T[j-1])/2

Device pipeline (per superchunk = 2048 tokens: an A-chunk from the first half
of the core's tokens and a B-chunk from the second half, pair-packed into 128
partitions):
  bcast:  xb2[128, 1024] f32 psum = one bf16 matmul.  lhsT E_blk[6,128] is a
          0/1 selector; rhs rows are an exact 3-way bf16 split of x
          (hi+mid+lo == x exactly, bf16 shares f32's exponent range), so the
          f32 psum accumulation reconstructs x EXACTLY on 128 partitions
          (rows 0:64 = x_A, 64:128 = x_B).  No fp32 matmuls anywhere.
  sign:   alternating engines per superchunk (this is the throughput-critical
          pair of passes; ACT and DVE each do one pass per superchunk):
            ACT:  sg = Sign(xb2 + (-low))            in {-1, 0, +1}
            DVE:  sg = (xb2 + (-low)) >= 0           in {0, 1}
          Both sign-exact (f32 add is correctly rounded; only exact ties are
          wrong, patched on host).
  gather: one 128-deep block-diag fp16 matmul: lhsT = blockdiag(V2, V2) for
          the +/-1 grid or 2*blockdiag(V2,V2) for the {0,1} grid.
  copy:   the other engine copies psum -> fp16 sbuf; for {0,1} superchunks the
          -T[63] correction (V2^T 1 = T[63]) rides along as a per-partition
          bias/add.
  out:    coalesced 256KB fp16 DMAs (two superchunks per [64, 4KB-rows] DMA).
Host transposes [D, NTOK] -> [NTOK, D], casts fp16 -> f32, and patches exact
bin-edge ties.
"""

import os as _os
import sys

import numpy as np

for _p in ("/opt/trn_rl_repo",):
    if _p not in sys.path:
        sys.path.insert(0, _p)

import ml_dtypes  # noqa: E402

import concourse.bass as bass  # noqa: E402,F401
import concourse.mybir as mybir  # noqa: E402
import concourse.tile as tile  # noqa: E402
from concourse import bacc  # noqa: E402
from concourse import bass_utils  # noqa: E402

B, F, K, D = 8192, 64, 64, 64
NCORES = 8
NTOK = (B // NCORES) * F          # 65536 tokens per core
CHUNK = 1024                      # tokens per chunk
NSUP = NTOK // (2 * CHUNK)        # 32 superchunks (A-chunk + B-chunk each)
NGRP = NSUP // 2                  # 16 groups of 2 superchunks (one out-DMA pair)
HALF = 512                        # matmul free dim (one psum bank of f32)

BF16 = mybir.dt.bfloat16
FP16 = mybir.dt.float16
F32 = mybir.dt.float32
NPBF16 = ml_dtypes.bfloat16

CFG = {
    # sign engine per superchunk: ACT when (p % 2 == flip) else DVE
    "flip": 0,
    # every act_both_mod-th superchunk, ACT does BOTH sign+copy (shifts work
    # from DVE to the slightly faster ACT). 0 = off.
    "act_both_mod": 0,
    # dummy matmuls at kernel start to trip the PE HAM throttle into 2.4 GHz
    # while the first input DMAs are still in flight (HAM appears pinned at
    # 1.2 GHz on this system, so default off)
    "warmup_mms": 0,
    # superchunks per output tile (each yields two out-DMAs of
    # sup_per_ot * 64KB each)
    "sup_per_ot": 4,
    # GPSIMD-broadcast superchunks per 8 (offloads the broadcast matmul from
    # the 1.2GHz-pinned PE to the otherwise idle GPSIMD engine)
    "gp8": 0,
    # superchunks of lead time for the slow gpsimd broadcast (~3.5us each)
    "gplead": 3,
    # copies go to ACT when p % copy_act_mod == 0 (else DVE)
    "copy_act_mod": 3,
}
for _kv in _os.environ.get("KCFG", "").split(","):
    if "=" in _kv:
        _k, _v = _kv.split("=", 1)
        CFG[_k.strip()] = int(_v) if _v.strip().lstrip("-").isdigit() else _v.strip()


_GP_SETS = {0: [], 1: [4], 2: [2, 6], 3: [1, 4, 6], 4: [0, 2, 4, 6]}


def _is_gp(p):
    return (p % 8) in _GP_SETS[CFG["gp8"]]


def _sign_is_act(p):
    if _is_gp(p):
        return False  # DVE: SBUF-src f32 tensor_scalar can hit 2x mode
    return True


def _copy_is_act(p):
    return (p % CFG["copy_act_mod"]) == 0


def build_tile_kernel(
    nc, tc, xs_d, xr_d, eblk_d, vblk_d, vgblk_d, neglow_d, negt63_d, out_d
):
    SPO = CFG["sup_per_ot"]                                       # superchunks per out tile
    OTW = SPO * CHUNK                                             # out tile width
    NREG = NSUP // SPO                                            # out regions per half
    xs_ap = xs_d.ap()                                             # [16, 6, 2048]
    xr_ap = xr_d.ap()                                             # [32, 2, 1024]
    out_ap = out_d.ap().rearrange("d (r n) -> r d n", r=2 * NREG)

    with tc.tile_pool(name="cpool", bufs=1) as cpool:
        eblk = cpool.tile([6, 128], BF16)
        nc.scalar.dma_start(out=eblk[:], in_=eblk_d.ap())
        vblk = cpool.tile([128, 128], FP16)
        nc.scalar.dma_start(out=vblk[:], in_=vblk_d.ap())
        vgblk = cpool.tile([128, 128], FP16)
        nc.scalar.dma_start(out=vgblk[:], in_=vgblk_d.ap())
        neglow = cpool.tile([128, 1], F32)
        nc.scalar.dma_start(out=neglow[:], in_=neglow_d.ap())
        negt63 = cpool.tile([128, 1], F32)
        nc.scalar.dma_start(out=negt63[:], in_=negt63_d.ap())
        dsrc = cpool.tile([64, 512], BF16)

        with (
            tc.tile_pool(name="xpool", bufs=3) as xpool,
            tc.tile_pool(name="xrpool", bufs=3) as xrpool,
            tc.tile_pool(name="sxpool", bufs=3) as sxpool,
            tc.tile_pool(name="spool", bufs=3) as spool,
            tc.tile_pool(name="opool", bufs=2) as opool,
            tc.tile_pool(name="pxpool", bufs=2, space="PSUM") as pxpool,
            tc.tile_pool(name="popool", bufs=2, space="PSUM") as popool,
        ):
            xs_t = {}
            xb2 = {}
            xbs = {}
            sg = {}
            ot = {}

            def stage_gp_prefetch(p):
                """GPSIMD broadcast path: DMA raw x rows, broadcast across
                partitions into SBUF (frees the PE of 2 matmuls)."""
                xr_t = xrpool.tile([1, 2 * CHUNK], F32, tag="xr", name="xr_t")
                nc.sync.dma_start(out=xr_t[:], in_=xr_ap[p])
                xb = sxpool.tile([128, CHUNK], F32, tag="xbs", name="xbs_t")
                nc.gpsimd.partition_broadcast(
                    xb[0:64, :], xr_t[0:1, 0:CHUNK], channels=64
                )
                nc.gpsimd.partition_broadcast(
                    xb[64:128, :], xr_t[0:1, CHUNK : 2 * CHUNK], channels=64
                )
                xbs[p] = xb

            # HAM warmup: keep the PE array continuously busy from t=0 so the
            # clock gate opens (1.2 -> 2.4 GHz) before the real matmuls start.
            if CFG["warmup_mms"]:
                nc.vector.memset(dsrc[:], 0.0)
                warm = pxpool.tile([128, CHUNK], F32, tag="xb", name="warm")
                for w in range(CFG["warmup_mms"]):
                    nc.tensor.matmul(
                        out=warm[0:64, HALF * (w % 2) : HALF * (w % 2 + 1)],
                        lhsT=dsrc[:, 0:64],
                        rhs=dsrc[:],
                        start=True,
                        stop=True,
                    )

            def stage_front(p):
                """DMA-in (per group), bcast matmul (or gpsimd result), sign."""
                g, half = divmod(p, 2)
                if half == 0:
                    xs_t[g] = xpool.tile([6, 2048], BF16, tag="xs", name="xs_t")
                    nc.sync.dma_start(out=xs_t[g][:], in_=xs_ap[g])
                if p % SPO == 0:
                    ot[p // SPO] = opool.tile([128, OTW], FP16, tag="ot", name="ot")
                if _is_gp(p):
                    xb = xbs.pop(p)
                else:
                    xb = pxpool.tile([128, CHUNK], F32, tag="xb")
                    for h in range(2):
                        nc.tensor.matmul(
                            out=xb[:, HALF * h : HALF * (h + 1)],
                            lhsT=eblk[:],
                            rhs=xs_t[g][:, CHUNK * half + HALF * h : CHUNK * half + HALF * (h + 1)],
                            start=True,
                            stop=True,
                        )
                    xb2[p] = xb
                s = spool.tile([128, CHUNK], FP16, tag="sg")
                if _sign_is_act(p):
                    nc.scalar.activation(
                        out=s[:],
                        in_=xb[:],
                        func=mybir.ActivationFunctionType.Sign,
                        bias=neglow[:],
                        scale=1.0,
                    )
                else:
                    nc.vector.tensor_scalar(
                        out=s[:],
                        in0=xb[:],
                        scalar1=neglow[:],
                        scalar2=0.0,
                        op0=mybir.AluOpType.add,
                        op1=mybir.AluOpType.is_ge,
                    )
                sg[p] = s

            def stage_back(p):
                """Gather matmul, psum->sbuf copy, out-DMA (per out tile)."""
                act_grid = _sign_is_act(p)
                table = vblk if act_grid else vgblk
                ps = popool.tile([128, CHUNK], F32, tag="ps")
                for h in range(2):
                    nc.tensor.matmul(
                        out=ps[:, HALF * h : HALF * (h + 1)],
                        lhsT=table[:],
                        rhs=sg[p][:, HALF * h : HALF * (h + 1)],
                        start=True,
                        stop=True,
                    )
                G, slot = divmod(p, SPO)
                dst = ot[G][:, CHUNK * slot : CHUNK * (slot + 1)]
                if _copy_is_act(p):
                    if act_grid:
                        nc.scalar.activation(
                            out=dst, in_=ps[:],
                            func=mybir.ActivationFunctionType.Copy,
                        )
                    else:
                        nc.scalar.activation(
                            out=dst, in_=ps[:],
                            func=mybir.ActivationFunctionType.Identity,
                            bias=negt63[:],
                            scale=1.0,
                        )
                else:
                    if act_grid:
                        nc.vector.tensor_copy(out=dst, in_=ps[:])
                    else:
                        nc.vector.tensor_scalar(
                            out=dst, in0=ps[:],
                            scalar1=negt63[:],
                            scalar2=None,
                            op0=mybir.AluOpType.add,
                        )
                del sg[p]
                xb2.pop(p, None)
                if slot == SPO - 1:
                    nc.sync.dma_start(out=out_ap[G], in_=ot[G][0:64, :])
                    nc.sync.dma_start(out=out_ap[NREG + G], in_=ot[G][64:128, :])

            # software pipeline: gp-broadcast prefetch runs gplead ahead,
            # front(p) runs one superchunk ahead of back(p)
            LEAD = CFG["gplead"]
            for p in range(NSUP + 1):
                if p == 0:
                    for q in range(min(LEAD, NSUP)):
                        if _is_gp(q):
                            stage_gp_prefetch(q)
                if p + LEAD < NSUP and _is_gp(p + LEAD):
                    stage_gp_prefetch(p + LEAD)
                if p < NSUP:
                    stage_front(p)
                if p >= 1:
                    stage_back(p - 1)


_CACHED_NC = None


def _get_nc():
    global _CACHED_NC
    if _CACHED_NC is None:
        nc = bacc.Bacc("TRN2", target_bir_lowering=False, debug=False)
        xs_d = nc.dram_tensor("xs", [NGRP, 6, 2048], BF16, kind="ExternalInput")
        xr_d = nc.dram_tensor("xr", [NSUP, 2 * CHUNK], F32, kind="ExternalInput")
        eblk_d = nc.dram_tensor("eblk", [6, 128], BF16, kind="ExternalInput")
        vblk_d = nc.dram_tensor("vblk", [128, 128], FP16, kind="ExternalInput")
        vgblk_d = nc.dram_tensor("vgblk", [128, 128], FP16, kind="ExternalInput")
        neglow_d = nc.dram_tensor("neglow", [128, 1], F32, kind="ExternalInput")
        negt63_d = nc.dram_tensor("negt63", [128, 1], F32, kind="ExternalInput")
        out_d = nc.dram_tensor("out", [D, NTOK], FP16, kind="ExternalOutput")
        with tile.TileContext(nc) as tc:
            build_tile_kernel(
                nc, tc, xs_d, xr_d, eblk_d, vblk_d, vgblk_d, neglow_d, negt63_d, out_d
            )
        nc.compile()
        _CACHED_NC = nc
    return _CACHED_NC


def make_host_tables(low, weight):
    """Constant device inputs, computed in float64."""
    ar = np.arange(K)
    S = 1.0 / (np.abs(ar[:, None] - ar[None, :]) + 1.0)              # [K, K] f64
    T = S @ np.asarray(weight, np.float64)                           # [K, D]
    V = np.empty_like(T)
    V[0] = (T[0] + T[-1]) / 2
    V[1:] = (T[1:] - T[:-1]) / 2

    vblk = np.zeros((128, 128), np.float64)
    vblk[0:64, 0:64] = V
    vblk[64:128, 64:128] = V
    vblk16 = vblk.astype(np.float16)
    vgblk16 = (2.0 * vblk).astype(np.float16)

    eblk = np.zeros((6, 128), np.float32)
    eblk[0:3, 0:64] = 1.0
    eblk[3:6, 64:128] = 1.0
    eblk16 = eblk.astype(NPBF16)

    lowfull = np.asarray(low, np.float64)                            # [-inf, bins]
    neg = np.where(np.isinf(lowfull), 3e38, -lowfull).astype(np.float32)
    neglow = np.concatenate([neg, neg]).reshape(128, 1)

    negt63 = np.concatenate([-T[63], -T[63]]).astype(np.float32).reshape(128, 1)
    return {
        "eblk": eblk16,
        "vblk": vblk16,
        "vgblk": vgblk16,
        "neglow": neglow,
        "negt63": negt63,
    }


def split_x_shard(shard):
    """Exact 3-way bf16 split of a [NTOK] f32 shard, arranged [NGRP, 6, 2048].

    Superchunk p pairs A-chunk p (tokens p*1024..) with B-chunk 32+p (tokens
    32768 + p*1024..).  Group g holds superchunks 2g (cols 0:1024) and 2g+1
    (cols 1024:2048); rows = hi/mid/lo of A then hi/mid/lo of B.
    """
    x = np.asarray(shard, np.float32)
    hi = x.astype(NPBF16).astype(np.float32)
    r = x - hi
    mid = r.astype(NPBF16).astype(np.float32)
    lo = (r - mid).astype(NPBF16)
    hi16 = hi.astype(NPBF16)
    mid16 = mid.astype(NPBF16)

    halfn = NTOK // 2
    parts = [hi16[:halfn], mid16[:halfn], lo[:halfn],
             hi16[halfn:], mid16[halfn:], lo[halfn:]]
    xs = np.empty((NGRP, 6, 2048), NPBF16)
    for r_i in range(6):
        # [32768] -> [16 groups, 2048 tokens] (natural order)
        xs[:, r_i, :] = parts[r_i].reshape(NGRP, 2048)
    return xs


def host_correct_ties(out2d, xflat, low, weight):
    """Exact fixup for tokens where x equals a bin edge: the device compare
    gives sign(0)=0 (ACT) or >=0 (DVE) there while the reference uses strict
    x > low. Replace those few rows with the exact table row."""
    bins = np.asarray(low, np.float32)[1:]
    ties = np.isin(xflat, bins)
    if not ties.any():
        return out2d
    xt = xflat[ties]
    idx = (xt[:, None] > bins[None, :]).sum(-1)
    ar = np.arange(K)
    S = 1.0 / (np.abs(ar[:, None] - ar[None, :]) + 1.0)
    T = (S @ np.asarray(weight, np.float64)).astype(np.float32)
    out2d[ties] = T[idx]
    return out2d


def build_in_maps(x, low, weight):
    consts = make_host_tables(low, weight)
    shards = np.asarray(x, np.float32).reshape(NCORES, NTOK)
    maps = []
    for i in range(NCORES):
        sh = shards[i]
        # xr[p] = [A-chunk p | B-chunk 32+p] raw f32 rows (gpsimd-broadcast path)
        halfn = NTOK // 2
        xr = np.concatenate(
            [sh[:halfn].reshape(NSUP, CHUNK), sh[halfn:].reshape(NSUP, CHUNK)],
            axis=1,
        )
        maps.append(
            {"xs": split_x_shard(sh), "xr": np.ascontiguousarray(xr), **consts}
        )
    return maps


def run_cores(x, low, weight, trace=False):
    """Shard, run on 8 cores, return ([NTOK*8, D] f32 output, BassKernelResults)."""
    nc = _get_nc()
    in_maps = build_in_maps(x, low, weight)
    res = bass_utils.run_bass_kernel_spmd(
        nc, in_maps, core_ids=list(range(NCORES)), trace=trace
    )
    out = np.concatenate(
        [
            np.ascontiguousarray(res.results[i]["out"].T).astype(np.float32)
            for i in range(NCORES)
        ],
        axis=0,
    )
    return out, res


def kernel(x, low, high, weight):
    x = np.asarray(x, np.float32)
    out, _ = run_cores(x, low, weight)
    out = host_correct_ties(out, x.reshape(-1), low, weight)
    return out.reshape(B, F, D)


# revision 20
# speedup vs baseline: 1.2111x; 1.2111x over previous
"""Trainium2 Bass kernel for nn_ContinuousEmbedding (histogram binning + distance-
weighted embedding mix).

Math: for each scalar x[b,f], the reference computes bucket index
idx = #{j in 1..63 : x > low[j]} and returns
    out[b,f,:] = sum_k weight[k,:] / (|idx-k|+1)  =  T[idx,:]
where T = S @ weight, S[i,k] = 1/(|i-k|+1) is a fixed 64x64 matrix.

T[idx] telescopes over compare signs s_j = sign(x - low[j]) (s_0 = +1 since
low[0] = -inf):
    T[idx] = sum_j s_j * V2[j],  V2[0] = (T[0]+T[63])/2, V2[j] = (T[j]-T[j-1])/2

Device pipeline (per superchunk = 2048 tokens: an A-chunk from the first half
of the core's tokens and a B-chunk from the second half, pair-packed into 128
partitions):
  bcast:  xb2[128, 1024] f32 psum = one bf16 matmul.  lhsT E_blk[6,128] is a
          0/1 selector; rhs rows are an exact 3-way bf16 split of x
          (hi+mid+lo == x exactly, bf16 shares f32's exponent range), so the
          f32 psum accumulation reconstructs x EXACTLY on 128 partitions
          (rows 0:64 = x_A, 64:128 = x_B).  No fp32 matmuls anywhere.
  sign:   alternating engines per superchunk (this is the throughput-critical
          pair of passes; ACT and DVE each do one pass per superchunk):
            ACT:  sg = Sign(xb2 + (-low))            in {-1, 0, +1}
            DVE:  sg = (xb2 + (-low)) >= 0           in {0, 1}
          Both sign-exact (f32 add is correctly rounded; only exact ties are
          wrong, patched on host).
  gather: one 128-deep block-diag fp16 matmul: lhsT = blockdiag(V2, V2) for
          the +/-1 grid or 2*blockdiag(V2,V2) for the {0,1} grid.
  copy:   the other engine copies psum -> fp16 sbuf; for {0,1} superchunks the
          -T[63] correction (V2^T 1 = T[63]) rides along as a per-partition
          bias/add.
  out:    coalesced 256KB fp16 DMAs (two superchunks per [64, 4KB-rows] DMA).
Host transposes [D, NTOK] -> [NTOK, D], casts fp16 -> f32, and patches exact
bin-edge ties.
"""

import os as _os
import sys

import numpy as np

for _p in ("/opt/trn_rl_repo",):
    if _p not in sys.path:
        sys.path.insert(0, _p)

import ml_dtypes  # noqa: E402

import concourse.bass as bass  # noqa: E402,F401
import concourse.mybir as mybir  # noqa: E402
import concourse.tile as tile  # noqa: E402
from concourse import bacc  # noqa: E402
from concourse import bass_utils  # noqa: E402

B, F, K, D = 8192, 64, 64, 64
NCORES = 8
NTOK = (B // NCORES) * F          # 65536 tokens per core
CHUNK = 1024                      # tokens per chunk
NSUP = NTOK // (2 * CHUNK)        # 32 superchunks (A-chunk + B-chunk each)
NGRP = NSUP // 2                  # 16 groups of 2 superchunks (one out-DMA pair)
HALF = 512                        # matmul free dim (one psum bank of f32)

BF16 = mybir.dt.bfloat16
FP16 = mybir.dt.float16
F32 = mybir.dt.float32
NPBF16 = ml_dtypes.bfloat16

CFG = {
    # sign engine per superchunk: ACT when (p % 2 == flip) else DVE
    "flip": 0,
    # every act_both_mod-th superchunk, ACT does BOTH sign+copy (shifts work
    # from DVE to the slightly faster ACT). 0 = off.
    "act_both_mod": 0,
    # dummy matmuls at kernel start to trip the PE HAM throttle into 2.4 GHz
    # while the first input DMAs are still in flight (HAM appears pinned at
    # 1.2 GHz on this system, so default off)
    "warmup_mms": 0,
    # superchunks per output tile (each yields two out-DMAs of
    # sup_per_ot * 64KB each)
    "sup_per_ot": 4,
    # GPSIMD-broadcast superchunks per 8 (offloads the broadcast matmul from
    # the 1.2GHz-pinned PE to the otherwise idle GPSIMD engine)
    "gp8": 0,
    # superchunks of lead time for the slow gpsimd broadcast (~3.5us each)
    "gplead": 3,
    # copies go to ACT when p % copy_act_mod == 0 (else DVE)
    "copy_act_mod": 3,
}
for _kv in _os.environ.get("KCFG", "").split(","):
    if "=" in _kv:
        _k, _v = _kv.split("=", 1)
        CFG[_k.strip()] = int(_v) if _v.strip().lstrip("-").isdigit() else _v.strip()


_GP_SETS = {0: [], 1: [4], 2: [2, 6], 3: [1, 4, 6], 4: [0, 2, 4, 6]}


def _is_gp(p):
    return (p % 8) in _GP_SETS[CFG["gp8"]]


def _sign_is_act(p):
    if _is_gp(p):
        return False
    return (p % 2) == CFG["flip"]


def _copy_is_act(p):
    if CFG["act_both_mod"] and (p % CFG["act_both_mod"]) == (CFG["act_both_mod"] - 1):
        return True
    return not _sign_is_act(p)


def build_tile_kernel(
    nc, tc, xs_d, xr_d, eblk_d, vblk_d, vgblk_d, neglow_d, negt63_d, out_d
):
    SPO = CFG["sup_per_ot"]                                       # superchunks per out tile
    OTW = SPO * CHUNK                                             # out tile width
    NREG = NSUP // SPO                                            # out regions per half
    xs_ap = xs_d.ap()                                             # [16, 6, 2048]
    xr_ap = xr_d.ap()                                             # [32, 2, 1024]
    out_ap = out_d.ap().rearrange("d (r n) -> r d n", r=2 * NREG)

    with tc.tile_pool(name="cpool", bufs=1) as cpool:
        eblk = cpool.tile([6, 128], BF16)
        nc.scalar.dma_start(out=eblk[:], in_=eblk_d.ap())
        vblk = cpool.tile([128, 128], FP16)
        nc.scalar.dma_start(out=vblk[:], in_=vblk_d.ap())
        vgblk = cpool.tile([128, 128], FP16)
        nc.scalar.dma_start(out=vgblk[:], in_=vgblk_d.ap())
        neglow = cpool.tile([128, 1], F32)
        nc.scalar.dma_start(out=neglow[:], in_=neglow_d.ap())
        negt63 = cpool.tile([128, 1], F32)
        nc.scalar.dma_start(out=negt63[:], in_=negt63_d.ap())
        dsrc = cpool.tile([64, 512], BF16)

        with (
            tc.tile_pool(name="xpool", bufs=3) as xpool,
            tc.tile_pool(name="xrpool", bufs=3) as xrpool,
            tc.tile_pool(name="sxpool", bufs=3) as sxpool,
            tc.tile_pool(name="spool", bufs=3) as spool,
            tc.tile_pool(name="opool", bufs=2) as opool,
            tc.tile_pool(name="pxpool", bufs=2, space="PSUM") as pxpool,
            tc.tile_pool(name="popool", bufs=2, space="PSUM") as popool,
        ):
            xs_t = {}
            xb2 = {}
            xbs = {}
            sg = {}
            ot = {}

            def stage_gp_prefetch(p):
                """GPSIMD broadcast path: DMA raw x rows, broadcast across
                partitions into SBUF (frees the PE of 2 matmuls)."""
                xr_t = xrpool.tile([1, 2 * CHUNK], F32, tag="xr", name="xr_t")
                nc.sync.dma_start(out=xr_t[:], in_=xr_ap[p])
                xb = sxpool.tile([128, CHUNK], F32, tag="xbs", name="xbs_t")
                nc.gpsimd.partition_broadcast(
                    xb[0:64, :], xr_t[0:1, 0:CHUNK], channels=64
                )
                nc.gpsimd.partition_broadcast(
                    xb[64:128, :], xr_t[0:1, CHUNK : 2 * CHUNK], channels=64
                )
                xbs[p] = xb

            # HAM warmup: keep the PE array continuously busy from t=0 so the
            # clock gate opens (1.2 -> 2.4 GHz) before the real matmuls start.
            if CFG["warmup_mms"]:
                nc.vector.memset(dsrc[:], 0.0)
                warm = pxpool.tile([128, CHUNK], F32, tag="xb", name="warm")
                for w in range(CFG["warmup_mms"]):
                    nc.tensor.matmul(
                        out=warm[0:64, HALF * (w % 2) : HALF * (w % 2 + 1)],
                        lhsT=dsrc[:, 0:64],
                        rhs=dsrc[:],
                        start=True,
                        stop=True,
                    )

            def stage_front(p):
                """DMA-in (per group), bcast matmul (or gpsimd result), sign."""
                g, half = divmod(p, 2)
                if half == 0:
                    xs_t[g] = xpool.tile([6, 2048], BF16, tag="xs", name="xs_t")
                    nc.sync.dma_start(out=xs_t[g][:], in_=xs_ap[g])
                if p % SPO == 0:
                    ot[p // SPO] = opool.tile([128, OTW], FP16, tag="ot", name="ot")
                if _is_gp(p):
                    xb = xbs.pop(p)
                else:
                    xb = pxpool.tile([128, CHUNK], F32, tag="xb")
                    for h in range(2):
                        nc.tensor.matmul(
                            out=xb[:, HALF * h : HALF * (h + 1)],
                            lhsT=eblk[:],
                            rhs=xs_t[g][:, CHUNK * half + HALF * h : CHUNK * half + HALF * (h + 1)],
                            start=True,
                            stop=True,
                        )
                    xb2[p] = xb
                s = spool.tile([128, CHUNK], FP16, tag="sg")
                if _sign_is_act(p):
                    nc.scalar.activation(
                        out=s[:],
                        in_=xb[:],
                        func=mybir.ActivationFunctionType.Sign,
                        bias=neglow[:],
                        scale=1.0,
                    )
                else:
                    nc.vector.tensor_scalar(
                        out=s[:],
                        in0=xb[:],
                        scalar1=neglow[:],
                        scalar2=0.0,
                        op0=mybir.AluOpType.add,
                        op1=mybir.AluOpType.is_ge,
                    )
                sg[p] = s

            def stage_back(p):
                """Gather matmul, psum->sbuf copy, out-DMA (per out tile)."""
                act_grid = _sign_is_act(p)
                table = vblk if act_grid else vgblk
                ps = popool.tile([128, CHUNK], F32, tag="ps")
                for h in range(2):
                    nc.tensor.matmul(
                        out=ps[:, HALF * h : HALF * (h + 1)],
                        lhsT=table[:],
                        rhs=sg[p][:, HALF * h : HALF * (h + 1)],
                        start=True,
                        stop=True,
                    )
                G, slot = divmod(p, SPO)
                dst = ot[G][:, CHUNK * slot : CHUNK * (slot + 1)]
                if _copy_is_act(p):
                    if act_grid:
                        nc.scalar.activation(
                            out=dst, in_=ps[:],
                            func=mybir.ActivationFunctionType.Copy,
                        )
                    else:
                        nc.scalar.activation(
                            out=dst, in_=ps[:],
                            func=mybir.ActivationFunctionType.Identity,
                            bias=negt63[:],
                            scale=1.0,
                        )
                else:
                    if act_grid:
                        nc.vector.tensor_copy(out=dst, in_=ps[:])
                    else:
                        nc.vector.tensor_scalar(
                            out=dst, in0=ps[:],
                            scalar1=negt63[:],
                            scalar2=None,
                            op0=mybir.AluOpType.add,
                        )
                del sg[p]
                xb2.pop(p, None)
                if slot == SPO - 1:
                    nc.sync.dma_start(out=out_ap[G], in_=ot[G][0:64, :])
                    nc.sync.dma_start(out=out_ap[NREG + G], in_=ot[G][64:128, :])

            # software pipeline: gp-broadcast prefetch runs gplead ahead,
            # front(p) runs one superchunk ahead of back(p)
            LEAD = CFG["gplead"]
            for p in range(NSUP + 1):
                if p == 0:
                    for q in range(min(LEAD, NSUP)):
                        if _is_gp(q):
                            stage_gp_prefetch(q)
                if p + LEAD < NSUP and _is_gp(p + LEAD):
                    stage_gp_prefetch(p + LEAD)
                if p < NSUP:
                    stage_front(p)
                if p >= 1:
                    stage_back(p - 1)


_CACHED_NC = None


def _get_nc():
    global _CACHED_NC
    if _CACHED_NC is None:
        nc = bacc.Bacc("TRN2", target_bir_lowering=False, debug=False)
        xs_d = nc.dram_tensor("xs", [NGRP, 6, 2048], BF16, kind="ExternalInput")
        xr_d = nc.dram_tensor("xr", [NSUP, 2 * CHUNK], F32, kind="ExternalInput")
        eblk_d = nc.dram_tensor("eblk", [6, 128], BF16, kind="ExternalInput")
        vblk_d = nc.dram_tensor("vblk", [128, 128], FP16, kind="ExternalInput")
        vgblk_d = nc.dram_tensor("vgblk", [128, 128], FP16, kind="ExternalInput")
        neglow_d = nc.dram_tensor("neglow", [128, 1], F32, kind="ExternalInput")
        negt63_d = nc.dram_tensor("negt63", [128, 1], F32, kind="ExternalInput")
        out_d = nc.dram_tensor("out", [D, NTOK], FP16, kind="ExternalOutput")
        with tile.TileContext(nc) as tc:
            build_tile_kernel(
                nc, tc, xs_d, xr_d, eblk_d, vblk_d, vgblk_d, neglow_d, negt63_d, out_d
            )
        nc.compile()
        _CACHED_NC = nc
    return _CACHED_NC


def make_host_tables(low, weight):
    """Constant device inputs, computed in float64."""
    ar = np.arange(K)
    S = 1.0 / (np.abs(ar[:, None] - ar[None, :]) + 1.0)              # [K, K] f64
    T = S @ np.asarray(weight, np.float64)                           # [K, D]
    V = np.empty_like(T)
    V[0] = (T[0] + T[-1]) / 2
    V[1:] = (T[1:] - T[:-1]) / 2

    vblk = np.zeros((128, 128), np.float64)
    vblk[0:64, 0:64] = V
    vblk[64:128, 64:128] = V
    vblk16 = vblk.astype(np.float16)
    vgblk16 = (2.0 * vblk).astype(np.float16)

    eblk = np.zeros((6, 128), np.float32)
    eblk[0:3, 0:64] = 1.0
    eblk[3:6, 64:128] = 1.0
    eblk16 = eblk.astype(NPBF16)

    lowfull = np.asarray(low, np.float64)                            # [-inf, bins]
    neg = np.where(np.isinf(lowfull), 3e38, -lowfull).astype(np.float32)
    neglow = np.concatenate([neg, neg]).reshape(128, 1)

    negt63 = np.concatenate([-T[63], -T[63]]).astype(np.float32).reshape(128, 1)
    return {
        "eblk": eblk16,
        "vblk": vblk16,
        "vgblk": vgblk16,
        "neglow": neglow,
        "negt63": negt63,
    }


def split_x_shard(shard):
    """Exact 3-way bf16 split of a [NTOK] f32 shard, arranged [NGRP, 6, 2048].

    Superchunk p pairs A-chunk p (tokens p*1024..) with B-chunk 32+p (tokens
    32768 + p*1024..).  Group g holds superchunks 2g (cols 0:1024) and 2g+1
    (cols 1024:2048); rows = hi/mid/lo of A then hi/mid/lo of B.
    """
    x = np.asarray(shard, np.float32)
    hi = x.astype(NPBF16).astype(np.float32)
    r = x - hi
    mid = r.astype(NPBF16).astype(np.float32)
    lo = (r - mid).astype(NPBF16)
    hi16 = hi.astype(NPBF16)
    mid16 = mid.astype(NPBF16)

    halfn = NTOK // 2
    parts = [hi16[:halfn], mid16[:halfn], lo[:halfn],
             hi16[halfn:], mid16[halfn:], lo[halfn:]]
    xs = np.empty((NGRP, 6, 2048), NPBF16)
    for r_i in range(6):
        # [32768] -> [16 groups, 2048 tokens] (natural order)
        xs[:, r_i, :] = parts[r_i].reshape(NGRP, 2048)
    return xs


def host_correct_ties(out2d, xflat, low, weight):
    """Exact fixup for tokens where x equals a bin edge: the device compare
    gives sign(0)=0 (ACT) or >=0 (DVE) there while the reference uses strict
    x > low. Replace those few rows with the exact table row."""
    bins = np.asarray(low, np.float32)[1:]
    ties = np.isin(xflat, bins)
    if not ties.any():
        return out2d
    xt = xflat[ties]
    idx = (xt[:, None] > bins[None, :]).sum(-1)
    ar = np.arange(K)
    S = 1.0 / (np.abs(ar[:, None] - ar[None, :]) + 1.0)
    T = (S @ np.asarray(weight, np.float64)).astype(np.float32)
    out2d[ties] = T[idx]
    return out2d


def build_in_maps(x, low, weight):
    consts = make_host_tables(low, weight)
    shards = np.asarray(x, np.float32).reshape(NCORES, NTOK)
    maps = []
    for i in range(NCORES):
        sh = shards[i]
        # xr[p] = [A-chunk p | B-chunk 32+p] raw f32 rows (gpsimd-broadcast path)
        halfn = NTOK // 2
        xr = np.concatenate(
            [sh[:halfn].reshape(NSUP, CHUNK), sh[halfn:].reshape(NSUP, CHUNK)],
            axis=1,
        )
        maps.append(
            {"xs": split_x_shard(sh), "xr": np.ascontiguousarray(xr), **consts}
        )
    return maps


def run_cores(x, low, weight, trace=False):
    """Shard, run on 8 cores, return ([NTOK*8, D] f32 output, BassKernelResults)."""
    nc = _get_nc()
    in_maps = build_in_maps(x, low, weight)
    res = bass_utils.run_bass_kernel_spmd(
        nc, in_maps, core_ids=list(range(NCORES)), trace=trace
    )
    out = np.concatenate(
        [
            np.ascontiguousarray(res.results[i]["out"].T).astype(np.float32)
            for i in range(NCORES)
        ],
        axis=0,
    )
    return out, res


def kernel(x, low, high, weight):
    x = np.asarray(x, np.float32)
    out, _ = run_cores(x, low, weight)
    out = host_correct_ties(out, x.reshape(-1), low, weight)
    return out.reshape(B, F, D)


# revision 22
# speedup vs baseline: 1.2958x; 1.0699x over previous
"""Trainium2 Bass kernel for nn_ContinuousEmbedding (histogram binning + distance-
weighted embedding mix).

Math: for each scalar x[b,f], the reference computes bucket index
idx = #{j in 1..63 : x > low[j]} and returns
    out[b,f,:] = sum_k weight[k,:] / (|idx-k|+1)  =  T[idx,:]
where T = S @ weight, S[i,k] = 1/(|i-k|+1) is a fixed 64x64 matrix.

T[idx] telescopes over compare signs s_j = sign(x - low[j]) (s_0 = +1 since
low[0] = -inf):
    T[idx] = sum_j s_j * V2[j],  V2[0] = (T[0]+T[63])/2, V2[j] = (T[j]-T[j-1])/2

Device pipeline (per superchunk = 2048 tokens: an A-chunk from the first half
of the core's tokens and a B-chunk from the second half, pair-packed into 128
partitions):
  bcast:  xb2[128, 1024] f32 psum = one bf16 matmul.  lhsT E_blk[6,128] is a
          0/1 selector; rhs rows are an exact 3-way bf16 split of x
          (hi+mid+lo == x exactly, bf16 shares f32's exponent range), so the
          f32 psum accumulation reconstructs x EXACTLY on 128 partitions
          (rows 0:64 = x_A, 64:128 = x_B).  No fp32 matmuls anywhere.
  sign:   alternating engines per superchunk (this is the throughput-critical
          pair of passes; ACT and DVE each do one pass per superchunk):
            ACT:  sg = Sign(xb2 + (-low))            in {-1, 0, +1}
            DVE:  sg = (xb2 + (-low)) >= 0           in {0, 1}
          Both sign-exact (f32 add is correctly rounded; only exact ties are
          wrong, patched on host).
  gather: one 128-deep block-diag fp16 matmul: lhsT = blockdiag(V2, V2) for
          the +/-1 grid or 2*blockdiag(V2,V2) for the {0,1} grid.
  copy:   the other engine copies psum -> fp16 sbuf; for {0,1} superchunks the
          -T[63] correction (V2^T 1 = T[63]) rides along as a per-partition
          bias/add.
  out:    coalesced 256KB fp16 DMAs (two superchunks per [64, 4KB-rows] DMA).
Host transposes [D, NTOK] -> [NTOK, D], casts fp16 -> f32, and patches exact
bin-edge ties.
"""

import os as _os
import sys

import numpy as np

for _p in ("/opt/trn_rl_repo",):
    if _p not in sys.path:
        sys.path.insert(0, _p)

import ml_dtypes  # noqa: E402

import concourse.bass as bass  # noqa: E402,F401
import concourse.mybir as mybir  # noqa: E402
import concourse.tile as tile  # noqa: E402
from concourse import bacc  # noqa: E402
from concourse import bass_utils  # noqa: E402

B, F, K, D = 8192, 64, 64, 64
NCORES = 8
NTOK = (B // NCORES) * F          # 65536 tokens per core
CHUNK = 1024                      # tokens per chunk
NSUP = NTOK // (2 * CHUNK)        # 32 superchunks (A-chunk + B-chunk each)
NGRP = NSUP // 2                  # 16 groups of 2 superchunks (one out-DMA pair)
HALF = 512                        # matmul free dim (one psum bank of f32)

BF16 = mybir.dt.bfloat16
FP16 = mybir.dt.float16
F32 = mybir.dt.float32
NPBF16 = ml_dtypes.bfloat16

CFG = {
    # sign engine per superchunk: ACT when (p % 2 == flip) else DVE
    "flip": 0,
    # every act_both_mod-th superchunk, ACT does BOTH sign+copy (shifts work
    # from DVE to the slightly faster ACT). 0 = off.
    "act_both_mod": 0,
    # dummy matmuls at kernel start to trip the PE HAM throttle into 2.4 GHz
    # while the first input DMAs are still in flight (HAM appears pinned at
    # 1.2 GHz on this system, so default off)
    "warmup_mms": 0,
    # superchunks per output tile (each yields two out-DMAs of
    # sup_per_ot * 64KB each)
    "sup_per_ot": 4,
    # GPSIMD-broadcast superchunks per 8 (offloads the broadcast matmul from
    # the 1.2GHz-pinned PE to the otherwise idle GPSIMD engine)
    "gp8": 0,
    # superchunks of lead time for the slow gpsimd broadcast (~3.5us each)
    "gplead": 3,
    # copies go to ACT when p % copy_act_mod == 0 (else DVE)
    "copy_act_mod": 3,
}
for _kv in _os.environ.get("KCFG", "").split(","):
    if "=" in _kv:
        _k, _v = _kv.split("=", 1)
        CFG[_k.strip()] = int(_v) if _v.strip().lstrip("-").isdigit() else _v.strip()


_GP_SETS = {0: [], 1: [4], 2: [2, 6], 3: [1, 4, 6], 4: [0, 2, 4, 6]}


def _is_gp(p):
    return (p % 8) in _GP_SETS[CFG["gp8"]]


def _sign_is_act(p):
    if _is_gp(p):
        return False
    return (p % 2) == CFG["flip"]


def _copy_is_act(p):
    if CFG["act_both_mod"] and (p % CFG["act_both_mod"]) == (CFG["act_both_mod"] - 1):
        return True
    return not _sign_is_act(p)


def build_tile_kernel(
    nc, tc, xs_d, xr_d, eblk_d, vblk_d, vgblk_d, neglow_d, negt63_d, out_d
):
    SPO = CFG["sup_per_ot"]                                       # superchunks per out tile
    OTW = SPO * CHUNK                                             # out tile width
    NREG = NSUP // SPO                                            # out regions per half
    xs_ap = xs_d.ap()                                             # [16, 6, 2048]
    xr_ap = xr_d.ap()                                             # [32, 2, 1024]
    out_ap = out_d.ap().rearrange("d (r n) -> r d n", r=2 * NREG)

    with tc.tile_pool(name="cpool", bufs=1) as cpool:
        eblk = cpool.tile([6, 128], BF16)
        nc.scalar.dma_start(out=eblk[:], in_=eblk_d.ap())
        vblk = cpool.tile([128, 128], FP16)
        nc.scalar.dma_start(out=vblk[:], in_=vblk_d.ap())
        vgblk = cpool.tile([128, 128], FP16)
        nc.scalar.dma_start(out=vgblk[:], in_=vgblk_d.ap())
        neglow = cpool.tile([128, 1], F32)
        nc.scalar.dma_start(out=neglow[:], in_=neglow_d.ap())
        negt63 = cpool.tile([128, 1], F32)
        nc.scalar.dma_start(out=negt63[:], in_=negt63_d.ap())
        dsrc = cpool.tile([64, 512], BF16)

        with (
            tc.tile_pool(name="xpool", bufs=4) as xpool,
            tc.tile_pool(name="xrpool", bufs=3) as xrpool,
            tc.tile_pool(name="sxpool", bufs=3) as sxpool,
            tc.tile_pool(name="spool", bufs=3) as spool,
            tc.tile_pool(name="opool", bufs=2) as opool,
            tc.tile_pool(name="pxpool", bufs=2, space="PSUM") as pxpool,
            tc.tile_pool(name="popool", bufs=2, space="PSUM") as popool,
        ):
            xs_t = {}
            xb2 = {}
            xbs = {}
            sg = {}
            ot = {}

            def stage_gp_prefetch(p):
                """GPSIMD broadcast path: DMA raw x rows, broadcast across
                partitions into SBUF (frees the PE of 2 matmuls)."""
                xr_t = xrpool.tile([1, 2 * CHUNK], F32, tag="xr", name="xr_t")
                nc.sync.dma_start(out=xr_t[:], in_=xr_ap[p])
                xb = sxpool.tile([128, CHUNK], F32, tag="xbs", name="xbs_t")
                nc.gpsimd.partition_broadcast(
                    xb[0:64, :], xr_t[0:1, 0:CHUNK], channels=64
                )
                nc.gpsimd.partition_broadcast(
                    xb[64:128, :], xr_t[0:1, CHUNK : 2 * CHUNK], channels=64
                )
                xbs[p] = xb

            # HAM warmup: keep the PE array continuously busy from t=0 so the
            # clock gate opens (1.2 -> 2.4 GHz) before the real matmuls start.
            if CFG["warmup_mms"]:
                nc.vector.memset(dsrc[:], 0.0)
                warm = pxpool.tile([128, CHUNK], F32, tag="xb", name="warm")
                for w in range(CFG["warmup_mms"]):
                    nc.tensor.matmul(
                        out=warm[0:64, HALF * (w % 2) : HALF * (w % 2 + 1)],
                        lhsT=dsrc[:, 0:64],
                        rhs=dsrc[:],
                        start=True,
                        stop=True,
                    )

            def load_xs(g):
                if g < NGRP and g not in xs_t:
                    xs_t[g] = xpool.tile([6, 2048], BF16, tag="xs", name="xs_t")
                    nc.sync.dma_start(out=xs_t[g][:], in_=xs_ap[g])

            def stage_front(p):
                """bcast matmul (or gpsimd result), sign; xs prefetched ahead."""
                g, half = divmod(p, 2)
                if half == 0:
                    load_xs(g)        # no-op when already prefetched
                    load_xs(g + 2)    # prefetch: stay ahead of out-DMA queueing
                if p % SPO == 0:
                    ot[p // SPO] = opool.tile([128, OTW], FP16, tag="ot", name="ot")
                if _is_gp(p):
                    xb = xbs.pop(p)
                else:
                    xb = pxpool.tile([128, CHUNK], F32, tag="xb")
                    for h in range(2):
                        nc.tensor.matmul(
                            out=xb[:, HALF * h : HALF * (h + 1)],
                            lhsT=eblk[:],
                            rhs=xs_t[g][:, CHUNK * half + HALF * h : CHUNK * half + HALF * (h + 1)],
                            start=True,
                            stop=True,
                        )
                    xb2[p] = xb
                s = spool.tile([128, CHUNK], FP16, tag="sg")
                if _sign_is_act(p):
                    nc.scalar.activation(
                        out=s[:],
                        in_=xb[:],
                        func=mybir.ActivationFunctionType.Sign,
                        bias=neglow[:],
                        scale=1.0,
                    )
                else:
                    nc.vector.tensor_scalar(
                        out=s[:],
                        in0=xb[:],
                        scalar1=neglow[:],
                        scalar2=0.0,
                        op0=mybir.AluOpType.add,
                        op1=mybir.AluOpType.is_ge,
                    )
                sg[p] = s

            def stage_back(p):
                """Gather matmul, psum->sbuf copy, out-DMA (per out tile)."""
                act_grid = _sign_is_act(p)
                table = vblk if act_grid else vgblk
                ps = popool.tile([128, CHUNK], F32, tag="ps")
                for h in range(2):
                    nc.tensor.matmul(
                        out=ps[:, HALF * h : HALF * (h + 1)],
                        lhsT=table[:],
                        rhs=sg[p][:, HALF * h : HALF * (h + 1)],
                        start=True,
                        stop=True,
                    )
                G, slot = divmod(p, SPO)
                dst = ot[G][:, CHUNK * slot : CHUNK * (slot + 1)]
                if _copy_is_act(p):
                    if act_grid:
                        nc.scalar.activation(
                            out=dst, in_=ps[:],
                            func=mybir.ActivationFunctionType.Copy,
                        )
                    else:
                        nc.scalar.activation(
                            out=dst, in_=ps[:],
                            func=mybir.ActivationFunctionType.Identity,
                            bias=negt63[:],
                            scale=1.0,
                        )
                else:
                    if act_grid:
                        nc.vector.tensor_copy(out=dst, in_=ps[:])
                    else:
                        nc.vector.tensor_scalar(
                            out=dst, in0=ps[:],
                            scalar1=negt63[:],
                            scalar2=None,
                            op0=mybir.AluOpType.add,
                        )
                del sg[p]
                xb2.pop(p, None)
                if slot == SPO - 1:
                    nc.sync.dma_start(out=out_ap[G], in_=ot[G][0:64, :])
                    nc.sync.dma_start(out=out_ap[NREG + G], in_=ot[G][64:128, :])

            # software pipeline: gp-broadcast prefetch runs gplead ahead,
            # front(p) runs one superchunk ahead of back(p)
            LEAD = CFG["gplead"]
            load_xs(0)
            load_xs(1)
            for p in range(NSUP + 1):
                if p == 0:
                    for q in range(min(LEAD, NSUP)):
                        if _is_gp(q):
                            stage_gp_prefetch(q)
                if p + LEAD < NSUP and _is_gp(p + LEAD):
                    stage_gp_prefetch(p + LEAD)
                if p < NSUP:
                    stage_front(p)
                if p >= 1:
                    stage_back(p - 1)


_CACHED_NC = None


def _get_nc():
    global _CACHED_NC
    if _CACHED_NC is None:
        nc = bacc.Bacc("TRN2", target_bir_lowering=False, debug=False)
        xs_d = nc.dram_tensor("xs", [NGRP, 6, 2048], BF16, kind="ExternalInput")
        xr_d = nc.dram_tensor("xr", [NSUP, 2 * CHUNK], F32, kind="ExternalInput")
        eblk_d = nc.dram_tensor("eblk", [6, 128], BF16, kind="ExternalInput")
        vblk_d = nc.dram_tensor("vblk", [128, 128], FP16, kind="ExternalInput")
        vgblk_d = nc.dram_tensor("vgblk", [128, 128], FP16, kind="ExternalInput")
        neglow_d = nc.dram_tensor("neglow", [128, 1], F32, kind="ExternalInput")
        negt63_d = nc.dram_tensor("negt63", [128, 1], F32, kind="ExternalInput")
        out_d = nc.dram_tensor("out", [D, NTOK], FP16, kind="ExternalOutput")
        with tile.TileContext(nc) as tc:
            build_tile_kernel(
                nc, tc, xs_d, xr_d, eblk_d, vblk_d, vgblk_d, neglow_d, negt63_d, out_d
            )
        nc.compile()
        _CACHED_NC = nc
    return _CACHED_NC


def make_host_tables(low, weight):
    """Constant device inputs, computed in float64."""
    ar = np.arange(K)
    S = 1.0 / (np.abs(ar[:, None] - ar[None, :]) + 1.0)              # [K, K] f64
    T = S @ np.asarray(weight, np.float64)                           # [K, D]
    V = np.empty_like(T)
    V[0] = (T[0] + T[-1]) / 2
    V[1:] = (T[1:] - T[:-1]) / 2

    vblk = np.zeros((128, 128), np.float64)
    vblk[0:64, 0:64] = V
    vblk[64:128, 64:128] = V
    vblk16 = vblk.astype(np.float16)
    vgblk16 = (2.0 * vblk).astype(np.float16)

    eblk = np.zeros((6, 128), np.float32)
    eblk[0:3, 0:64] = 1.0
    eblk[3:6, 64:128] = 1.0
    eblk16 = eblk.astype(NPBF16)

    lowfull = np.asarray(low, np.float64)                            # [-inf, bins]
    neg = np.where(np.isinf(lowfull), 3e38, -lowfull).astype(np.float32)
    neglow = np.concatenate([neg, neg]).reshape(128, 1)

    negt63 = np.concatenate([-T[63], -T[63]]).astype(np.float32).reshape(128, 1)
    return {
        "eblk": eblk16,
        "vblk": vblk16,
        "vgblk": vgblk16,
        "neglow": neglow,
        "negt63": negt63,
    }


def split_x_shard(shard):
    """Exact 3-way bf16 split of a [NTOK] f32 shard, arranged [NGRP, 6, 2048].

    Superchunk p pairs A-chunk p (tokens p*1024..) with B-chunk 32+p (tokens
    32768 + p*1024..).  Group g holds superchunks 2g (cols 0:1024) and 2g+1
    (cols 1024:2048); rows = hi/mid/lo of A then hi/mid/lo of B.
    """
    x = np.asarray(shard, np.float32)
    hi = x.astype(NPBF16).astype(np.float32)
    r = x - hi
    mid = r.astype(NPBF16).astype(np.float32)
    lo = (r - mid).astype(NPBF16)
    hi16 = hi.astype(NPBF16)
    mid16 = mid.astype(NPBF16)

    halfn = NTOK // 2
    parts = [hi16[:halfn], mid16[:halfn], lo[:halfn],
             hi16[halfn:], mid16[halfn:], lo[halfn:]]
    xs = np.empty((NGRP, 6, 2048), NPBF16)
    for r_i in range(6):
        # [32768] -> [16 groups, 2048 tokens] (natural order)
        xs[:, r_i, :] = parts[r_i].reshape(NGRP, 2048)
    return xs


def host_correct_ties(out2d, xflat, low, weight):
    """Exact fixup for tokens where x equals a bin edge: the device compare
    gives sign(0)=0 (ACT) or >=0 (DVE) there while the reference uses strict
    x > low. Replace those few rows with the exact table row."""
    bins = np.asarray(low, np.float32)[1:]
    ties = np.isin(xflat, bins)
    if not ties.any():
        return out2d
    xt = xflat[ties]
    idx = (xt[:, None] > bins[None, :]).sum(-1)
    ar = np.arange(K)
    S = 1.0 / (np.abs(ar[:, None] - ar[None, :]) + 1.0)
    T = (S @ np.asarray(weight, np.float64)).astype(np.float32)
    out2d[ties] = T[idx]
    return out2d


def build_in_maps(x, low, weight):
    consts = make_host_tables(low, weight)
    shards = np.asarray(x, np.float32).reshape(NCORES, NTOK)
    maps = []
    for i in range(NCORES):
        sh = shards[i]
        # xr[p] = [A-chunk p | B-chunk 32+p] raw f32 rows (gpsimd-broadcast path)
        halfn = NTOK // 2
        xr = np.concatenate(
            [sh[:halfn].reshape(NSUP, CHUNK), sh[halfn:].reshape(NSUP, CHUNK)],
            axis=1,
        )
        maps.append(
            {"xs": split_x_shard(sh), "xr": np.ascontiguousarray(xr), **consts}
        )
    return maps


def run_cores(x, low, weight, trace=False):
    """Shard, run on 8 cores, return ([NTOK*8, D] f32 output, BassKernelResults)."""
    nc = _get_nc()
    in_maps = build_in_maps(x, low, weight)
    res = bass_utils.run_bass_kernel_spmd(
        nc, in_maps, core_ids=list(range(NCORES)), trace=trace
    )
    out = np.concatenate(
        [
            np.ascontiguousarray(res.results[i]["out"].T).astype(np.float32)
            for i in range(NCORES)
        ],
        axis=0,
    )
    return out, res


def kernel(x, low, high, weight):
    x = np.asarray(x, np.float32)
    out, _ = run_cores(x, low, weight)
    out = host_correct_ties(out, x.reshape(-1), low, weight)
    return out.reshape(B, F, D)


# revision 24
# speedup vs baseline: 1.3237x; 1.0216x over previous
"""Trainium2 Bass kernel for nn_ContinuousEmbedding (histogram binning + distance-
weighted embedding mix).

Math: for each scalar x[b,f], the reference computes bucket index
idx = #{j in 1..63 : x > low[j]} and returns
    out[b,f,:] = sum_k weight[k,:] / (|idx-k|+1)  =  T[idx,:]
where T = S @ weight, S[i,k] = 1/(|i-k|+1) is a fixed 64x64 matrix.

T[idx] telescopes over compare signs s_j = sign(x - low[j]) (s_0 = +1 since
low[0] = -inf):
    T[idx] = sum_j s_j * V2[j],  V2[0] = (T[0]+T[63])/2, V2[j] = (T[j]-T[j-1])/2

Device pipeline (per superchunk = 2048 tokens: an A-chunk from the first half
of the core's tokens and a B-chunk from the second half, pair-packed into 128
partitions):
  bcast:  xb2[128, 1024] f32 psum = one bf16 matmul.  lhsT E_blk[6,128] is a
          0/1 selector; rhs rows are an exact 3-way bf16 split of x
          (hi+mid+lo == x exactly, bf16 shares f32's exponent range), so the
          f32 psum accumulation reconstructs x EXACTLY on 128 partitions
          (rows 0:64 = x_A, 64:128 = x_B).  No fp32 matmuls anywhere.
  sign:   alternating engines per superchunk (this is the throughput-critical
          pair of passes; ACT and DVE each do one pass per superchunk):
            ACT:  sg = Sign(xb2 + (-low))            in {-1, 0, +1}
            DVE:  sg = (xb2 + (-low)) >= 0           in {0, 1}
          Both sign-exact (f32 add is correctly rounded; only exact ties are
          wrong, patched on host).
  gather: one 128-deep block-diag fp16 matmul: lhsT = blockdiag(V2, V2) for
          the +/-1 grid or 2*blockdiag(V2,V2) for the {0,1} grid.
  copy:   the other engine copies psum -> fp16 sbuf; for {0,1} superchunks the
          -T[63] correction (V2^T 1 = T[63]) rides along as a per-partition
          bias/add.
  out:    coalesced 256KB fp16 DMAs (two superchunks per [64, 4KB-rows] DMA).
Host transposes [D, NTOK] -> [NTOK, D], casts fp16 -> f32, and patches exact
bin-edge ties.
"""

import os as _os
import sys

import numpy as np

for _p in ("/opt/trn_rl_repo",):
    if _p not in sys.path:
        sys.path.insert(0, _p)

import ml_dtypes  # noqa: E402

import concourse.bass as bass  # noqa: E402,F401
import concourse.mybir as mybir  # noqa: E402
import concourse.tile as tile  # noqa: E402
from concourse import bacc  # noqa: E402
from concourse import bass_utils  # noqa: E402

B, F, K, D = 8192, 64, 64, 64
NCORES = 8
NTOK = (B // NCORES) * F          # 65536 tokens per core
CHUNK = 1024                      # tokens per chunk
NSUP = NTOK // (2 * CHUNK)        # 32 superchunks (A-chunk + B-chunk each)
NGRP = NSUP // 2                  # 16 groups of 2 superchunks (one out-DMA pair)
HALF = 512                        # matmul free dim (one psum bank of f32)

BF16 = mybir.dt.bfloat16
FP16 = mybir.dt.float16
F32 = mybir.dt.float32
NPBF16 = ml_dtypes.bfloat16

CFG = {
    # sign engine per superchunk: ACT when (p % 2 == flip) else DVE
    "flip": 0,
    # every act_both_mod-th superchunk, ACT does BOTH sign+copy (shifts work
    # from DVE to the slightly faster ACT). 0 = off.
    "act_both_mod": 0,
    # dummy matmuls at kernel start to trip the PE HAM throttle into 2.4 GHz
    # while the first input DMAs are still in flight (HAM appears pinned at
    # 1.2 GHz on this system, so default off)
    "warmup_mms": 0,
    # superchunks per output tile (each yields two out-DMAs of
    # sup_per_ot * 64KB each)
    "sup_per_ot": 4,
    # GPSIMD-broadcast superchunks per 8 (offloads the broadcast matmul from
    # the 1.2GHz-pinned PE to the otherwise idle GPSIMD engine)
    "gp8": 0,
    # superchunks of lead time for the slow gpsimd broadcast (~3.5us each)
    "gplead": 3,
    # copies go to ACT when p % copy_act_mod == 0 (else DVE)
    "copy_act_mod": 3,
}
for _kv in _os.environ.get("KCFG", "").split(","):
    if "=" in _kv:
        _k, _v = _kv.split("=", 1)
        CFG[_k.strip()] = int(_v) if _v.strip().lstrip("-").isdigit() else _v.strip()


_GP_SETS = {0: [], 1: [4], 2: [2, 6], 3: [1, 4, 6], 4: [0, 2, 4, 6]}


def _is_gp(p):
    return (p % 8) in _GP_SETS[CFG["gp8"]]


def _sign_is_act(p):
    if _is_gp(p):
        return False
    return (p % 2) == CFG["flip"]


def _copy_is_act(p):
    if CFG["act_both_mod"] and (p % CFG["act_both_mod"]) == (CFG["act_both_mod"] - 1):
        return True
    return not _sign_is_act(p)


def build_tile_kernel(
    nc, tc, xs_d, xr_d, eblk_d, vblk_d, vgblk_d, neglow_d, negt63_d, out_d
):
    SPO = CFG["sup_per_ot"]                                       # superchunks per out tile
    OTW = SPO * CHUNK                                             # out tile width
    NREG = NSUP // SPO                                            # out regions per half
    xs_ap = xs_d.ap()                                             # [16, 6, 2048]
    xr_ap = xr_d.ap()                                             # [32, 2, 1024]
    out_ap = out_d.ap().rearrange("d (r n) -> r d n", r=2 * NREG)

    with tc.tile_pool(name="cpool", bufs=1) as cpool:
        eblk = cpool.tile([6, 128], BF16)
        nc.scalar.dma_start(out=eblk[:], in_=eblk_d.ap())
        vblk = cpool.tile([128, 128], FP16)
        nc.scalar.dma_start(out=vblk[:], in_=vblk_d.ap())
        vgblk = cpool.tile([128, 128], FP16)
        nc.scalar.dma_start(out=vgblk[:], in_=vgblk_d.ap())
        neglow = cpool.tile([128, 1], F32)
        nc.scalar.dma_start(out=neglow[:], in_=neglow_d.ap())
        negt63 = cpool.tile([128, 1], F32)
        nc.scalar.dma_start(out=negt63[:], in_=negt63_d.ap())
        dsrc = cpool.tile([64, 512], BF16)

        with (
            tc.tile_pool(name="xpool", bufs=4) as xpool,
            tc.tile_pool(name="xrpool", bufs=3) as xrpool,
            tc.tile_pool(name="sxpool", bufs=3) as sxpool,
            tc.tile_pool(name="spool", bufs=3) as spool,
            tc.tile_pool(name="opool", bufs=2) as opool,
            tc.tile_pool(name="pxpool", bufs=2, space="PSUM") as pxpool,
            tc.tile_pool(name="popool", bufs=2, space="PSUM") as popool,
        ):
            xs_t = {}
            xb2 = {}
            xbs = {}
            sg = {}
            ot = {}

            def stage_gp_prefetch(p):
                """GPSIMD broadcast path: DMA raw x rows, broadcast across
                partitions into SBUF (frees the PE of 2 matmuls)."""
                xr_t = xrpool.tile([1, 2 * CHUNK], F32, tag="xr", name="xr_t")
                nc.sync.dma_start(out=xr_t[:], in_=xr_ap[p])
                xb = sxpool.tile([128, CHUNK], F32, tag="xbs", name="xbs_t")
                nc.gpsimd.partition_broadcast(
                    xb[0:64, :], xr_t[0:1, 0:CHUNK], channels=64
                )
                nc.gpsimd.partition_broadcast(
                    xb[64:128, :], xr_t[0:1, CHUNK : 2 * CHUNK], channels=64
                )
                xbs[p] = xb

            # HAM warmup: keep the PE array continuously busy from t=0 so the
            # clock gate opens (1.2 -> 2.4 GHz) before the real matmuls start.
            if CFG["warmup_mms"]:
                nc.vector.memset(dsrc[:], 0.0)
                warm = pxpool.tile([128, CHUNK], F32, tag="xb", name="warm")
                for w in range(CFG["warmup_mms"]):
                    nc.tensor.matmul(
                        out=warm[0:64, HALF * (w % 2) : HALF * (w % 2 + 1)],
                        lhsT=dsrc[:, 0:64],
                        rhs=dsrc[:],
                        start=True,
                        stop=True,
                    )

            def load_xs(g):
                if g < NGRP and g not in xs_t:
                    xs_t[g] = xpool.tile([6, 2048], BF16, tag="xs", name="xs_t")
                    nc.sync.dma_start(out=xs_t[g][:], in_=xs_ap[g])

            def stage_front(p):
                """bcast matmul (or gpsimd result), sign; xs prefetched ahead."""
                g, half = divmod(p, 2)
                if half == 0:
                    load_xs(g)        # no-op when already prefetched
                    load_xs(g + 2)    # prefetch: stay ahead of out-DMA queueing
                if p % SPO == 0:
                    ot[p // SPO] = opool.tile([128, OTW], FP16, tag="ot", name="ot")
                if _is_gp(p):
                    xb = xbs.pop(p)
                else:
                    xb = pxpool.tile([128, CHUNK], F32, tag="xb")
                    for h in range(2):
                        nc.tensor.matmul(
                            out=xb[:, HALF * h : HALF * (h + 1)],
                            lhsT=eblk[:],
                            rhs=xs_t[g][:, CHUNK * half + HALF * h : CHUNK * half + HALF * (h + 1)],
                            start=True,
                            stop=True,
                        )
                    xb2[p] = xb
                s = spool.tile([128, CHUNK], FP16, tag="sg")
                if _sign_is_act(p):
                    nc.scalar.activation(
                        out=s[:],
                        in_=xb[:],
                        func=mybir.ActivationFunctionType.Sign,
                        bias=neglow[:],
                        scale=1.0,
                    )
                else:
                    nc.vector.tensor_scalar(
                        out=s[:],
                        in0=xb[:],
                        scalar1=neglow[:],
                        scalar2=0.0,
                        op0=mybir.AluOpType.add,
                        op1=mybir.AluOpType.is_ge,
                    )
                sg[p] = s

            def stage_back(p):
                """Gather matmul, psum->sbuf copy, out-DMA (per out tile)."""
                act_grid = _sign_is_act(p)
                table = vblk if act_grid else vgblk
                ps = popool.tile([128, CHUNK], F32, tag="ps")
                for h in range(2):
                    nc.tensor.matmul(
                        out=ps[:, HALF * h : HALF * (h + 1)],
                        lhsT=table[:],
                        rhs=sg[p][:, HALF * h : HALF * (h + 1)],
                        start=True,
                        stop=True,
                    )
                G, slot = divmod(p, SPO)
                dst = ot[G][:, CHUNK * slot : CHUNK * (slot + 1)]
                if _copy_is_act(p):
                    if act_grid:
                        nc.scalar.activation(
                            out=dst, in_=ps[:],
                            func=mybir.ActivationFunctionType.Copy,
                        )
                    else:
                        nc.scalar.activation(
                            out=dst, in_=ps[:],
                            func=mybir.ActivationFunctionType.Identity,
                            bias=negt63[:],
                            scale=1.0,
                        )
                else:
                    if act_grid:
                        nc.vector.tensor_copy(out=dst, in_=ps[:])
                    else:
                        nc.vector.tensor_scalar(
                            out=dst, in0=ps[:],
                            scalar1=negt63[:],
                            scalar2=None,
                            op0=mybir.AluOpType.add,
                        )
                del sg[p]
                xb2.pop(p, None)
                if G == NREG - 1 and SPO >= 2:
                    # last group: flush in halves so the final DMA (which the
                    # kernel end waits on) is small
                    HW_ = OTW // 2
                    if slot == SPO // 2 - 1:
                        nc.sync.dma_start(
                            out=out_ap[G][:, 0:HW_], in_=ot[G][0:64, 0:HW_]
                        )
                        nc.sync.dma_start(
                            out=out_ap[NREG + G][:, 0:HW_], in_=ot[G][64:128, 0:HW_]
                        )
                    elif slot == SPO - 1:
                        nc.sync.dma_start(
                            out=out_ap[G][:, HW_:OTW], in_=ot[G][0:64, HW_:OTW]
                        )
                        nc.sync.dma_start(
                            out=out_ap[NREG + G][:, HW_:OTW], in_=ot[G][64:128, HW_:OTW]
                        )
                elif slot == SPO - 1:
                    nc.sync.dma_start(out=out_ap[G], in_=ot[G][0:64, :])
                    nc.sync.dma_start(out=out_ap[NREG + G], in_=ot[G][64:128, :])

            # software pipeline in 2-superchunk phases: front(2q, 2q+1) emits
            # bc x4 then signs; back of the previous phase emits ga x4 then
            # copies. Grouping the bc/ga matmuls halves the PE stationary
            # switches and gives each sign a full phase of slack before its
            # gather needs it.
            LEAD = CFG["gplead"]
            load_xs(0)
            load_xs(1)
            for q in range(NGRP + 1):
                for p in (2 * q, 2 * q + 1):
                    if p + LEAD < NSUP and _is_gp(p + LEAD):
                        stage_gp_prefetch(p + LEAD)
                    if q == 0 and _is_gp(p) and p < LEAD:
                        stage_gp_prefetch(p)
                if q < NGRP:
                    stage_front(2 * q)
                    stage_front(2 * q + 1)
                if q >= 1:
                    stage_back(2 * q - 2)
                    stage_back(2 * q - 1)


_CACHED_NC = None


def _get_nc():
    global _CACHED_NC
    if _CACHED_NC is None:
        nc = bacc.Bacc("TRN2", target_bir_lowering=False, debug=False)
        xs_d = nc.dram_tensor("xs", [NGRP, 6, 2048], BF16, kind="ExternalInput")
        xr_d = nc.dram_tensor("xr", [NSUP, 2 * CHUNK], F32, kind="ExternalInput")
        eblk_d = nc.dram_tensor("eblk", [6, 128], BF16, kind="ExternalInput")
        vblk_d = nc.dram_tensor("vblk", [128, 128], FP16, kind="ExternalInput")
        vgblk_d = nc.dram_tensor("vgblk", [128, 128], FP16, kind="ExternalInput")
        neglow_d = nc.dram_tensor("neglow", [128, 1], F32, kind="ExternalInput")
        negt63_d = nc.dram_tensor("negt63", [128, 1], F32, kind="ExternalInput")
        out_d = nc.dram_tensor("out", [D, NTOK], FP16, kind="ExternalOutput")
        with tile.TileContext(nc) as tc:
            build_tile_kernel(
                nc, tc, xs_d, xr_d, eblk_d, vblk_d, vgblk_d, neglow_d, negt63_d, out_d
            )
        nc.compile()
        _CACHED_NC = nc
    return _CACHED_NC


def make_host_tables(low, weight):
    """Constant device inputs, computed in float64."""
    ar = np.arange(K)
    S = 1.0 / (np.abs(ar[:, None] - ar[None, :]) + 1.0)              # [K, K] f64
    T = S @ np.asarray(weight, np.float64)                           # [K, D]
    V = np.empty_like(T)
    V[0] = (T[0] + T[-1]) / 2
    V[1:] = (T[1:] - T[:-1]) / 2

    vblk = np.zeros((128, 128), np.float64)
    vblk[0:64, 0:64] = V
    vblk[64:128, 64:128] = V
    vblk16 = vblk.astype(np.float16)
    vgblk16 = (2.0 * vblk).astype(np.float16)

    eblk = np.zeros((6, 128), np.float32)
    eblk[0:3, 0:64] = 1.0
    eblk[3:6, 64:128] = 1.0
    eblk16 = eblk.astype(NPBF16)

    lowfull = np.asarray(low, np.float64)                            # [-inf, bins]
    neg = np.where(np.isinf(lowfull), 3e38, -lowfull).astype(np.float32)
    neglow = np.concatenate([neg, neg]).reshape(128, 1)

    negt63 = np.concatenate([-T[63], -T[63]]).astype(np.float32).reshape(128, 1)
    return {
        "eblk": eblk16,
        "vblk": vblk16,
        "vgblk": vgblk16,
        "neglow": neglow,
        "negt63": negt63,
    }


def split_x_shard(shard):
    """Exact 3-way bf16 split of a [NTOK] f32 shard, arranged [NGRP, 6, 2048].

    Superchunk p pairs A-chunk p (tokens p*1024..) with B-chunk 32+p (tokens
    32768 + p*1024..).  Group g holds superchunks 2g (cols 0:1024) and 2g+1
    (cols 1024:2048); rows = hi/mid/lo of A then hi/mid/lo of B.
    """
    x = np.asarray(shard, np.float32)
    hi = x.astype(NPBF16).astype(np.float32)
    r = x - hi
    mid = r.astype(NPBF16).astype(np.float32)
    lo = (r - mid).astype(NPBF16)
    hi16 = hi.astype(NPBF16)
    mid16 = mid.astype(NPBF16)

    halfn = NTOK // 2
    parts = [hi16[:halfn], mid16[:halfn], lo[:halfn],
             hi16[halfn:], mid16[halfn:], lo[halfn:]]
    xs = np.empty((NGRP, 6, 2048), NPBF16)
    for r_i in range(6):
        # [32768] -> [16 groups, 2048 tokens] (natural order)
        xs[:, r_i, :] = parts[r_i].reshape(NGRP, 2048)
    return xs


def host_correct_ties(out2d, xflat, low, weight):
    """Exact fixup for tokens where x equals a bin edge: the device compare
    gives sign(0)=0 (ACT) or >=0 (DVE) there while the reference uses strict
    x > low. Replace those few rows with the exact table row."""
    bins = np.asarray(low, np.float32)[1:]
    ties = np.isin(xflat, bins)
    if not ties.any():
        return out2d
    xt = xflat[ties]
    idx = (xt[:, None] > bins[None, :]).sum(-1)
    ar = np.arange(K)
    S = 1.0 / (np.abs(ar[:, None] - ar[None, :]) + 1.0)
    T = (S @ np.asarray(weight, np.float64)).astype(np.float32)
    out2d[ties] = T[idx]
    return out2d


def build_in_maps(x, low, weight):
    consts = make_host_tables(low, weight)
    shards = np.asarray(x, np.float32).reshape(NCORES, NTOK)
    maps = []
    for i in range(NCORES):
        sh = shards[i]
        # xr[p] = [A-chunk p | B-chunk 32+p] raw f32 rows (gpsimd-broadcast path)
        halfn = NTOK // 2
        xr = np.concatenate(
            [sh[:halfn].reshape(NSUP, CHUNK), sh[halfn:].reshape(NSUP, CHUNK)],
            axis=1,
        )
        maps.append(
            {"xs": split_x_shard(sh), "xr": np.ascontiguousarray(xr), **consts}
        )
    return maps


def run_cores(x, low, weight, trace=False):
    """Shard, run on 8 cores, return ([NTOK*8, D] f32 output, BassKernelResults)."""
    nc = _get_nc()
    in_maps = build_in_maps(x, low, weight)
    res = bass_utils.run_bass_kernel_spmd(
        nc, in_maps, core_ids=list(range(NCORES)), trace=trace
    )
    out = np.concatenate(
        [
            np.ascontiguousarray(res.results[i]["out"].T).astype(np.float32)
            for i in range(NCORES)
        ],
        axis=0,
    )
    return out, res


def kernel(x, low, high, weight):
    x = np.asarray(x, np.float32)
    out, _ = run_cores(x, low, weight)
    out = host_correct_ties(out, x.reshape(-1), low, weight)
    return out.reshape(B, F, D)
